# revision 19
# baseline (speedup 1.0000x reference)
"""Trainium2 Bass kernel for nn_Def_A2C_Sample_Generator.

Computation (see reference):
  x = concat(state, payoff, noise)            (500, 504)
  h1 = lrelu(bn(adj @ (x @ w1) + b1))         (500, 32)
  h2 = lrelu(bn(adj @ (h1 @ w2) + b2))        (500, 16)
  xf = h2.reshape(8000)
  logits = xf @ actgen_w + def_cur_loc @ actgen_v          (50, 500)
  out = softmax(logits[None] + gumbel(u), axis=-1)         (1000, 50, 500)

Sharding: data-parallel over the 1000 samples, 125 per core on 8
cores. Each core computes the logits redundantly and softmaxes its own
125 x 50 x 500 gumbel block.

Softmax factoring: with v = ln(u) (ONE chunk-wide ACT pass),
  exp(logits + g) = L * exp(g) = L * (-1/v),  L = exp(logits)
and the minus sign cancels in the softmax normalization, so
  out = (L/v) / sum_t(L/v)
  v    : Ln ACT pass, chunk-wide in-place          (scalar engine)
  a    : reciprocal_approx_fast(v) in-place        (DVE, ~51 ULP)
  L    : PE ones-matmul broadcast of the fp16 L row into PSUM
  q,S  : DVE scalar_tensor_tensor mult, fused row-sum accum
  out  : gpsimd normalize_recip (out = q / S)      (GPSIMD engine)

DMA model (measured): each queue retires descriptors at ~160ns
regardless of size, fanned over ~5 HW engines (16 for the gpsimd SW
queue) -- so DESCRIPTOR COUNT, not bytes, is the currency. Layout:
  sync ring   : f32 param pack (1 DMA, 126 x 16.5KB descriptors),
                small params, then 5 u mega-chunk loads ([125, 5000]
                f32 = 20KB contiguous per partition).
  scalar ring : f16 param pack, logits bounce, 5 mega-chunk stores
                (20KB descriptors).
  gpsimd ring : wr stream x16 (8KB descriptors; wpool=12 keeps the
                first 12 MB.. tiles un-gated so the z matmuls chase the
                stream without recycle stalls).
All [500,*] DRAM params ship packed 4-rows-per-partition (partition p
holds rows 4p..4p+3 contiguously); matmuls use stride-4 stationary
column slices so K-groupings of stationary and moving operands match.
"""
import sys

if "/opt/trn_rl_repo" not in sys.path:
    sys.path.insert(0, "/opt/trn_rl_repo")

import numpy as np

import concourse.bacc as bacc
import concourse.bass as bass
import concourse.mybir as mybir
import concourse.tile as tile
from concourse import bass_utils

# Keep every Exp and Ln on act-table set 6 (natural_log_exp_and_others)
# so one table load serves both and nothing thrashes.
_orig_get_act_tables = bacc.get_activation_tables


def _patched_get_act_tables(arch):
    tabs = dict(_orig_get_act_tables(arch))
    both = {mybir.ActivationFunctionType.Exp, mybir.ActivationFunctionType.Ln}
    for name, fns in tabs.items():
        if name != "natural_log_exp_and_others" and (both & fns):
            tabs[name] = fns - both
    return tabs


bacc.get_activation_tables = _patched_get_act_tables

F32 = mybir.dt.float32
F16 = mybir.dt.float16
BF16 = mybir.dt.bfloat16
NCORES = 8
T = 500
R = 50
NS = 1000
SP = NS // NCORES  # 125 samples per core
H1, H2 = 32, 16
FIN = 504
KT = 4
NEG_SLOPE = 0.2
CH = 10  # r's per chunk (20KB contiguous per partition per DMA)
CW = CH * T
NCHUNK = R // CH  # 10
FRONT = 2  # chunks 0-1 front-run; rest emitted lazily in-loop

# f32 param pack column offsets: xT | adjT | w1
PK_XT, PK_ADJ, PK_W1 = 0, KT * T, 2 * KT * T
PK32_W = 2 * KT * T + KT * H1  # 4128
# f16 param pack: av | dclT
PK_AV, PK_DCL = 0, KT * T
PK16_W = KT * T + KT * R  # 2200

_CACHE = {}


def _build():
    nc = bacc.Bacc("TRN2", target_bir_lowering=False, debug=False,
                   enable_asserts=False, num_devices=NCORES)

    din = {}
    din["pk32"] = nc.dram_tensor("pk32", [126, PK32_W], BF16,
                                 kind="ExternalInput")
    din["pk16"] = nc.dram_tensor("pk16", [125, PK16_W], F16,
                                 kind="ExternalInput")
    din["b1"] = nc.dram_tensor("b1", [1, H1], BF16, kind="ExternalInput")
    din["w2"] = nc.dram_tensor("w2", [H1, H2], BF16, kind="ExternalInput")
    din["b2"] = nc.dram_tensor("b2", [1, H2], BF16, kind="ExternalInput")
    din["grow"] = nc.dram_tensor("grow", [1, T], BF16, kind="ExternalInput")
    din["brow"] = nc.dram_tensor("brow", [1, T], BF16, kind="ExternalInput")
    din["ident"] = nc.dram_tensor("ident", [H2, H2], BF16,
                                  kind="ExternalInput")
    din["wr"] = nc.dram_tensor("wr", [H2, 125, KT * T], BF16,
                               kind="ExternalInput")
    din["u"] = nc.dram_tensor("u", [SP, R, T], F32, kind="ExternalInput")
    out = nc.dram_tensor("out", [SP, R, T], F16, kind="ExternalOutput")

    with tile.TileContext(nc) as tc:
        _emit(nc, tc, din, out)
    nc.compile()
    return nc


def _emit(nc, tc, din, out):
    from contextlib import ExitStack

    ctx = ExitStack()
    with ctx:
        # ---------- pools ----------
        const = ctx.enter_context(tc.tile_pool(name="const", bufs=1))
        small = ctx.enter_context(tc.tile_pool(name="small", bufs=1))
        psum = ctx.enter_context(tc.tile_pool(name="psum", bufs=1,
                                              space="PSUM"))
        dram = ctx.enter_context(tc.tile_pool(name="dram", bufs=1,
                                              space="DRAM"))
        upool = ctx.enter_context(tc.tile_pool(name="upool", bufs=3))
        opool = ctx.enter_context(tc.tile_pool(name="opool", bufs=3))
        qpool = ctx.enter_context(tc.tile_pool(name="qpool", bufs=3))
        spool = ctx.enter_context(tc.tile_pool(name="spool", bufs=8))
        bppool = ctx.enter_context(tc.tile_pool(name="bppool", bufs=4,
                                                space="PSUM"))
        wpool = ctx.enter_context(tc.tile_pool(name="wpool", bufs=16))

        # ---------- params: 2 packed DMAs + small stragglers ----------
        pk32 = const.tile([126, PK32_W], BF16, tag="pk32", name="pk32")
        nc.gpsimd.dma_start(pk32[:], din["pk32"][:])
        pk16 = const.tile([125, PK16_W], F16, tag="pk16", name="pk16")
        nc.sync.dma_start(pk16[:], din["pk16"][:])
        b1 = const.tile([1, H1], BF16, tag="b1", name="b1")
        nc.sync.dma_start(b1[:], din["b1"][:])
        w2 = const.tile([H1, H2], BF16, tag="w2", name="w2")
        nc.sync.dma_start(w2[:], din["w2"][:])
        b2 = const.tile([1, H2], BF16, tag="b2", name="b2")
        nc.sync.dma_start(b2[:], din["b2"][:])
        grow = const.tile([1, T], BF16, tag="grow", name="grow")
        nc.sync.dma_start(grow[:], din["grow"][:])
        brow = const.tile([1, T], BF16, tag="brow", name="brow")
        nc.sync.dma_start(brow[:], din["brow"][:])
        ident = const.tile([H2, H2], BF16, tag="ident", name="ident")
        nc.sync.dma_start(ident[:], din["ident"][:])

        def xT(k, g=None):  # f-block k, target group g (cols 4m+g)
            base = PK_XT + k * T
            if g is None:
                return pk32[:, base:base + T]
            return pk32[:, base + g:base + T:KT]

        def adjT(k):
            base = PK_ADJ + k * T
            return pk32[0:125, base:base + T]

        def w1(k):
            base = PK_W1 + k * H1
            return pk32[:, base:base + H1]

        def av(k):
            return pk16[:, PK_AV + k * T:PK_AV + (k + 1) * T]

        def dclT(k):
            return pk16[:, PK_DCL + k * R:PK_DCL + (k + 1) * R]

        # ---------- wr stream: 16 un-gated DMAs emitted right after the
        # params. The global DMA semaphore pool (~11 sems, recycled in
        # emission order) makes DMA #n wait for #(n-11)'s completion, so
        # the u loads emitted after these auto-cascade in as wr DMAs
        # retire -- scheduler-proof bandwidth priority for the z path. --
        wts = []
        for c in range(H2):
            wt = wpool.tile([125, KT * T], BF16, tag="wr", name="wr")
            nc.gpsimd.dma_start(wt[:], din["wr"][c])
            wts.append(wt)

        # ---------- u loads: SAME ring as wr, directly behind it. The
        # gpsimd ring's sem pool (~8, recycled in ring order) makes u_k's
        # issue wait for wr DMA #(k+9)'s completion -- scheduler-proof
        # in-ring priority: the z path gets the fabric first, u cascades
        # in as wr retires. u7-9 are emitted lazily in the loop bodies
        # (their recycle waits resolve instantly there). ----
        uts = []

        def load_u(ci):
            ut = upool.tile([SP, CW], F32, tag="u", name="u")
            nc.gpsimd.dma_start(
                ut[:].rearrange("p (c t) -> p c t", c=CH),
                din["u"][:, ci * CH:(ci + 1) * CH, :])
            uts.append(ut)

        for ci in range(FRONT):
            load_u(ci)

        ones = const.tile([65, 128], BF16, tag="ones", name="ones")
        nc.vector.memset(ones[:], 1.0)
        ones16 = const.tile([65, 128], F16, tag="ones16", name="ones16")
        nc.vector.memset(ones16[:], 1.0)

        # ---------- gumbel front-run ----------
        def gumbel_chunk(ci):
            ut = uts[ci]
            nc.scalar.activation(ut[:], ut[:],
                                 mybir.ActivationFunctionType.Ln)
            nc.vector.reciprocal_approx_fast(ut[:], ut[:])

        for ci in range(FRONT):
            gumbel_chunk(ci)

        # ---------- GCN (bn folded into adjT; K-groups are 4p+k) ----
        def lrelu_from_psum(ps_ap, out_tile, width):
            tmp = small.tile([width, T], F32, tag=f"lr{width}",
                             name=f"lr{width}")
            nc.vector.tensor_scalar_mul(tmp[:], ps_ap, NEG_SLOPE)
            nc.vector.tensor_tensor(out_tile[:], tmp[:], ps_ap,
                                    op=mybir.AluOpType.max)

        xw1 = [small.tile([125, H1], BF16, tag=f"xw1{g}", name=f"xw1{g}")
               for g in range(KT)]
        for g in range(KT):
            ps = psum.tile([125, H1], F32, tag="ps_small", name="ps_small")
            for k in range(KT):
                nc.tensor.matmul(ps[:], xT(k, g), w1(k),
                                 start=(k == 0), stop=(k == KT - 1))
            nc.vector.tensor_copy(xw1[g][:], ps[:])

        a1ps = psum.tile([H1, T], F32, tag="ps_small", name="ps_small")
        for k in range(KT):
            nc.tensor.matmul(a1ps[:], xw1[k][:], adjT(k),
                             start=(k == 0), stop=False)
        nc.tensor.matmul(a1ps[:], b1[:], grow[:], start=False, stop=False)
        nc.tensor.matmul(a1ps[:], ones[0:1, :H1], brow[:], start=False,
                         stop=True)
        h1T = small.tile([H1, T], BF16, tag="h1T", name="h1T")
        lrelu_from_psum(a1ps[:], h1T, H1)

        xw2 = [small.tile([125, H2], BF16, tag=f"xw2{g}", name=f"xw2{g}")
               for g in range(KT)]
        for g in range(KT):
            ps = psum.tile([125, H2], F32, tag="ps_small", name="ps_small")
            nc.tensor.matmul(ps[:], h1T[:, g::KT], w2[:],
                             start=True, stop=True)
            nc.vector.tensor_copy(xw2[g][:], ps[:])

        a2ps = psum.tile([H2, T], F32, tag="ps_small", name="ps_small")
        for k in range(KT):
            nc.tensor.matmul(a2ps[:], xw2[k][:], adjT(k),
                             start=(k == 0), stop=False)
        nc.tensor.matmul(a2ps[:], b2[:], grow[:], start=False, stop=False)
        nc.tensor.matmul(a2ps[:], ones[0:1, :H2], brow[:], start=False,
                         stop=True)
        h2T = small.tile([H2, T], BF16, tag="h2T", name="h2T")
        lrelu_from_psum(a2ps[:], h2T, H2)

        h2b = [small.tile([125, H2], BF16, tag=f"h2b{k}", name=f"h2b{k}")
               for k in range(KT)]
        for k in range(KT):
            pt = psum.tile([125, H2], BF16, tag="ps_tr", name="ps_tr")
            nc.tensor.transpose(pt[:], h2T[:, k::KT], ident[:])
            nc.vector.tensor_copy(h2b[k][:], pt[:])

        # ---------- z = xf @ actgen_w (bf16 stream) ----------
        zps = psum.tile([1, T], F32, tag="ps_z", name="ps_z")
        first = True
        for c in range(H2):
            wt = wts[c]
            for k in range(KT):
                nc.tensor.matmul(zps[:], h2b[k][:, c:c + 1],
                                 wt[:, k * T:(k + 1) * T],
                                 start=first,
                                 stop=(c == H2 - 1 and k == KT - 1))
                first = False
        zrow = small.tile([1, T], F32, tag="zrow", name="zrow")
        nc.vector.tensor_copy(zrow[:], zps[:])
        zrow16 = small.tile([1, T], F16, tag="zrow16", name="zrow16")
        nc.vector.tensor_copy(zrow16[:], zrow[:])

        # ---------- logits;  L = exp(logits) in fp16 ----------
        lgp = psum.tile([R, T], F32, tag="ps_lg", name="ps_lg")
        for k in range(KT):
            nc.tensor.matmul(lgp[:], dclT(k), av(k),
                             start=(k == 0), stop=False)
        nc.tensor.matmul(lgp[:], ones16[0:1, :R], zrow16[:], start=False,
                         stop=True)
        logits = small.tile([R, T], F32, tag="logits", name="logits")
        nc.scalar.activation(logits[:], lgp[:],
                             mybir.ActivationFunctionType.Exp)
        lg16 = small.tile([R, T], F16, tag="lg16", name="lg16")
        nc.vector.tensor_copy(lg16[:], logits[:])

        # Pack the 50 L rows into 3 lanes at base partitions 0/32/64 (the
        # only legal matmul operand bases) via a DRAM bounce (scalar ring).
        LPL = 17
        # bounce rides the sync ring: its sems retired with the small
        # params long ago, so these fire the moment exp lands instead of
        # queueing behind the u cascade on the gpsimd ring
        ld = dram.tile([R, T], F16, name="ldram")
        nc.sync.dma_start(ld[:], lg16[:])
        lgflat = small.tile([65, LPL * T], F16, tag="lgflat", name="lgflat")
        nc.sync.dma_start(
            lgflat[0:33:32, :].rearrange("l (j t) -> l j t", j=LPL),
            ld[0:2 * LPL].rearrange("(l j) t -> l j t", l=2))
        nc.sync.dma_start(
            lgflat[64:65, :(R - 2 * LPL) * T],
            ld[2 * LPL:R].rearrange("(o j) t -> o (j t)", o=1))

        def lg_slice(r):
            lane, j = r // LPL, r % LPL
            return (lgflat[lane * 32:lane * 32 + 1, j * T:(j + 1) * T],
                    ones16[lane * 32:lane * 32 + 1, :SP])

        # ---------- main sampling loop ----------
        for ci in range(NCHUNK):
            ut = uts[ci]
            # prefetch chunk ci+FRONT now, so its DMA is in flight while
            # this chunk computes (~2 staggered u loads resident)
            if ci + FRONT < NCHUNK:
                load_u(ci + FRONT)
                gumbel_chunk(ci + FRONT)
            ot = opool.tile([SP, CW], F16, tag="o", name="o")
            for g in range(CH):
                r = ci * CH + g
                seg = slice(g * T, (g + 1) * T)
                rhs, lhs_ones = lg_slice(r)
                bt = bppool.tile([SP, 512], F32, tag="bp", name="bp")
                nc.tensor.matmul(bt[:, :T], lhs_ones, rhs,
                                 start=True, stop=True)
                qt = qpool.tile([SP, T], F32, tag="q", name="q")
                ss = spool.tile([SP, 1], F32, tag="ss", name="ss")
                nc.vector.scalar_tensor_tensor(
                    qt[:], bt[:, :T], 0.0, ut[:, seg],
                    op0=mybir.AluOpType.bypass, op1=mybir.AluOpType.mult,
                    accum_out=ss[:])
                nc.gpsimd.normalize_recip(ot[:, seg], qt[:], ss[:])
            # store rides the gpsimd ring right behind this chunk's
            # normalizes (producer order); lazy u issues keep the sync
            # ring's buf-recycle waits out of everyone's way
            # stores ride the sync ring (idle after the bounce; its sems
            # are long retired) so they never perturb the u cascade
            nc.sync.dma_start(out[:, ci * CH:(ci + 1) * CH, :],
                                ot[:].rearrange("p (c t) -> p c t", c=CH))


def _get_nc():
    if "nc" not in _CACHE:
        _CACHE["nc"] = _build()
    return _CACHE["nc"]


def prep_in_maps(inputs):
    f32 = np.float32
    f16 = np.float16
    import ml_dtypes
    state = np.asarray(inputs["state"], f32)[0]
    payoff = np.asarray(inputs["payoff"], f32)
    noise = np.asarray(inputs["feat_noise"], f32)[0]
    xT = np.concatenate([state, payoff, noise], axis=1).T  # (504, 500)
    gamma = np.asarray(inputs["bn_gamma"], f32)
    beta = np.asarray(inputs["bn_beta"], f32)
    adjT = (np.asarray(inputs["norm_adj"], f32) * gamma[:, None]).T
    dclT = np.asarray(inputs["def_cur_loc"], f32).T      # (500, 50)
    av = np.asarray(inputs["actgen_v"], f32)             # (500, 500)
    wr_full = np.asarray(inputs["actgen_w"], f32).reshape(T, H2, T)
    wr_full = np.ascontiguousarray(wr_full.transpose(1, 0, 2))

    def pack4(a, p):  # [4p, N] -> [p, 4N]; partition row i holds 4i..4i+3
        n = a.shape[1]
        return np.ascontiguousarray(a).reshape(p, 4 * n)

    bf16 = ml_dtypes.bfloat16
    pk32 = np.zeros((126, PK32_W), bf16)
    pk32[:, PK_XT:PK_XT + KT * T] = pack4(xT, 126).astype(bf16)
    pk32[:125, PK_ADJ:PK_ADJ + KT * T] = pack4(adjT, 125).astype(bf16)
    pk32[:, PK_W1:PK_W1 + KT * H1] = pack4(
        np.asarray(inputs["gc1_w"], f32), 126).astype(bf16)
    pk16 = np.zeros((125, PK16_W), f16)
    pk16[:, PK_AV:PK_AV + KT * T] = pack4(av, 125).astype(f16)
    pk16[:, PK_DCL:PK_DCL + KT * R] = pack4(dclT, 125).astype(f16)

    common = {
        "pk32": pk32,
        "pk16": pk16,
        "b1": np.asarray(inputs["gc1_b"], f32).reshape(1, H1).astype(bf16),
        "w2": np.asarray(inputs["gc2_w"], f32).astype(bf16),
        "b2": np.asarray(inputs["gc2_b"], f32).reshape(1, H2).astype(bf16),
        "grow": gamma.reshape(1, T).astype(bf16),
        "brow": beta.reshape(1, T).astype(bf16),
        "ident": np.eye(H2).astype(bf16),
        "wr": np.ascontiguousarray(
            wr_full.reshape(H2, 125, KT * T)).astype(ml_dtypes.bfloat16),
    }
    u = np.asarray(inputs["gumbel_u"], f32)
    in_maps = []
    for i in range(NCORES):
        m = dict(common)
        m["u"] = np.ascontiguousarray(u[i * SP:(i + 1) * SP])
        in_maps.append(m)
    return in_maps


def run(inputs, trace=False):
    nc = _get_nc()
    in_maps = prep_in_maps(inputs)
    res = bass_utils.run_bass_kernel_spmd(
        nc, in_maps, core_ids=list(range(NCORES)), trace=trace)
    full = np.concatenate([res.results[i]["out"] for i in range(NCORES)],
                          axis=0).astype(np.float32)
    return full, res


def kernel(**inputs):
    full, _ = run(inputs)
    return full


# revision 21
# speedup vs baseline: 1.0768x; 1.0768x over previous
"""Trainium2 Bass kernel for nn_Def_A2C_Sample_Generator.

Computation (see reference):
  x = concat(state, payoff, noise)            (500, 504)
  h1 = lrelu(bn(adj @ (x @ w1) + b1))         (500, 32)
  h2 = lrelu(bn(adj @ (h1 @ w2) + b2))        (500, 16)
  xf = h2.reshape(8000)
  logits = xf @ actgen_w + def_cur_loc @ actgen_v          (50, 500)
  out = softmax(logits[None] + gumbel(u), axis=-1)         (1000, 50, 500)

Sharding: data-parallel over the 1000 samples, 125 per core on 8
cores. Each core computes the logits redundantly and softmaxes its own
125 x 50 x 500 gumbel block.

Softmax factoring: with v = ln(u) (ONE chunk-wide ACT pass),
  exp(logits + g) = L * exp(g) = L * (-1/v),  L = exp(logits)
and the minus sign cancels in the softmax normalization, so
  out = (L/v) / sum_t(L/v)
  v    : Ln ACT pass, chunk-wide in-place          (scalar engine)
  a    : reciprocal_approx_fast(v) in-place        (DVE, ~51 ULP)
  L    : PE ones-matmul broadcast of the fp16 L row into PSUM
  q,S  : DVE scalar_tensor_tensor mult, fused row-sum accum
  out  : gpsimd normalize_recip (out = q / S)      (GPSIMD engine)

DMA model (measured): each queue retires descriptors at ~160ns
regardless of size, fanned over ~5 HW engines (16 for the gpsimd SW
queue) -- so DESCRIPTOR COUNT, not bytes, is the currency. Layout:
  sync ring   : f32 param pack (1 DMA, 126 x 16.5KB descriptors),
                small params, then 5 u mega-chunk loads ([125, 5000]
                f32 = 20KB contiguous per partition).
  scalar ring : f16 param pack, logits bounce, 5 mega-chunk stores
                (20KB descriptors).
  gpsimd ring : wr stream x16 (8KB descriptors; wpool=12 keeps the
                first 12 MB.. tiles un-gated so the z matmuls chase the
                stream without recycle stalls).
All [500,*] DRAM params ship packed 4-rows-per-partition (partition p
holds rows 4p..4p+3 contiguously); matmuls use stride-4 stationary
column slices so K-groupings of stationary and moving operands match.
"""
import sys

if "/opt/trn_rl_repo" not in sys.path:
    sys.path.insert(0, "/opt/trn_rl_repo")

import numpy as np

import concourse.bacc as bacc
import concourse.bass as bass
import concourse.mybir as mybir
import concourse.tile as tile
from concourse import bass_utils

# Keep every Exp and Ln on act-table set 6 (natural_log_exp_and_others)
# so one table load serves both and nothing thrashes.
_orig_get_act_tables = bacc.get_activation_tables


def _patched_get_act_tables(arch):
    tabs = dict(_orig_get_act_tables(arch))
    both = {mybir.ActivationFunctionType.Exp, mybir.ActivationFunctionType.Ln}
    for name, fns in tabs.items():
        if name != "natural_log_exp_and_others" and (both & fns):
            tabs[name] = fns - both
    return tabs


bacc.get_activation_tables = _patched_get_act_tables

F32 = mybir.dt.float32
F16 = mybir.dt.float16
BF16 = mybir.dt.bfloat16
NCORES = 8
T = 500
R = 50
NS = 1000
SP = NS // NCORES  # 125 samples per core
H1, H2 = 32, 16
FIN = 504
KT = 4
NEG_SLOPE = 0.2
CH = 10  # r's per chunk (20KB contiguous per partition per DMA)
CW = CH * T
NCHUNK = R // CH  # 10
FRONT = 2  # chunks 0-1 front-run; rest emitted lazily in-loop

# f32 param pack column offsets: xT | adjT | w1
PK_XT, PK_ADJ, PK_W1 = 0, KT * T, 2 * KT * T
PK32_W = 2 * KT * T + KT * H1  # 4128
# f16 param pack: av | dclT
PK_AV, PK_DCL = 0, KT * T
PK16_W = KT * T + KT * R  # 2200

_CACHE = {}


def _build():
    nc = bacc.Bacc("TRN2", target_bir_lowering=False, debug=False,
                   enable_asserts=False, num_devices=NCORES)

    din = {}
    din["pk32"] = nc.dram_tensor("pk32", [126, PK32_W], BF16,
                                 kind="ExternalInput")
    din["pk16"] = nc.dram_tensor("pk16", [125, PK16_W], F16,
                                 kind="ExternalInput")
    din["b1"] = nc.dram_tensor("b1", [1, H1], BF16, kind="ExternalInput")
    din["w2"] = nc.dram_tensor("w2", [H1, H2], BF16, kind="ExternalInput")
    din["b2"] = nc.dram_tensor("b2", [1, H2], BF16, kind="ExternalInput")
    din["grow"] = nc.dram_tensor("grow", [1, T], BF16, kind="ExternalInput")
    din["brow"] = nc.dram_tensor("brow", [1, T], BF16, kind="ExternalInput")
    din["ident"] = nc.dram_tensor("ident", [H2, H2], BF16,
                                  kind="ExternalInput")
    din["wr"] = nc.dram_tensor("wr", [H2 // 2, 125, 2 * KT * T], BF16,
                               kind="ExternalInput")
    din["u"] = nc.dram_tensor("u", [SP, R, T], F32, kind="ExternalInput")
    out = nc.dram_tensor("out", [SP, R, T], F16, kind="ExternalOutput")

    with tile.TileContext(nc) as tc:
        _emit(nc, tc, din, out)
    nc.compile()
    return nc


def _emit(nc, tc, din, out):
    from contextlib import ExitStack

    ctx = ExitStack()
    with ctx:
        # ---------- pools ----------
        const = ctx.enter_context(tc.tile_pool(name="const", bufs=1))
        small = ctx.enter_context(tc.tile_pool(name="small", bufs=1))
        psum = ctx.enter_context(tc.tile_pool(name="psum", bufs=1,
                                              space="PSUM"))
        dram = ctx.enter_context(tc.tile_pool(name="dram", bufs=1,
                                              space="DRAM"))
        upool = ctx.enter_context(tc.tile_pool(name="upool", bufs=3))
        opool = ctx.enter_context(tc.tile_pool(name="opool", bufs=3))
        qpool = ctx.enter_context(tc.tile_pool(name="qpool", bufs=3))
        spool = ctx.enter_context(tc.tile_pool(name="spool", bufs=8))
        bppool = ctx.enter_context(tc.tile_pool(name="bppool", bufs=4,
                                                space="PSUM"))
        wpool = ctx.enter_context(tc.tile_pool(name="wpool", bufs=8))

        # ---------- params: 2 packed DMAs + small stragglers ----------
        pk32 = const.tile([126, PK32_W], BF16, tag="pk32", name="pk32")
        nc.gpsimd.dma_start(pk32[:], din["pk32"][:])
        pk16 = const.tile([125, PK16_W], F16, tag="pk16", name="pk16")
        nc.scalar.dma_start(pk16[:], din["pk16"][:])
        b1 = const.tile([1, H1], BF16, tag="b1", name="b1")
        nc.scalar.dma_start(b1[:], din["b1"][:])
        w2 = const.tile([H1, H2], BF16, tag="w2", name="w2")
        nc.scalar.dma_start(w2[:], din["w2"][:])
        b2 = const.tile([1, H2], BF16, tag="b2", name="b2")
        nc.scalar.dma_start(b2[:], din["b2"][:])
        grow = const.tile([1, T], BF16, tag="grow", name="grow")
        nc.scalar.dma_start(grow[:], din["grow"][:])
        brow = const.tile([1, T], BF16, tag="brow", name="brow")
        nc.scalar.dma_start(brow[:], din["brow"][:])
        ident = const.tile([H2, H2], BF16, tag="ident", name="ident")
        nc.scalar.dma_start(ident[:], din["ident"][:])

        def xT(k, g=None):  # f-block k, target group g (cols 4m+g)
            base = PK_XT + k * T
            if g is None:
                return pk32[:, base:base + T]
            return pk32[:, base + g:base + T:KT]

        def adjT(k):
            base = PK_ADJ + k * T
            return pk32[0:125, base:base + T]

        def w1(k):
            base = PK_W1 + k * H1
            return pk32[:, base:base + H1]

        def av(k):
            return pk16[:, PK_AV + k * T:PK_AV + (k + 1) * T]

        def dclT(k):
            return pk16[:, PK_DCL + k * R:PK_DCL + (k + 1) * R]

        # ---------- wr stream: 16 un-gated DMAs emitted right after the
        # params. The global DMA semaphore pool (~11 sems, recycled in
        # emission order) makes DMA #n wait for #(n-11)'s completion, so
        # the u loads emitted after these auto-cascade in as wr DMAs
        # retire -- scheduler-proof bandwidth priority for the z path. --
        wts = []
        for g in range(H2 // 2):
            wt = wpool.tile([125, 2 * KT * T], BF16, tag="wr", name="wr")
            nc.gpsimd.dma_start(wt[:], din["wr"][g])
            wts.append(wt)

        # ---------- u loads: SAME ring as wr, directly behind it. The
        # gpsimd ring's sem pool (~8, recycled in ring order) makes u_k's
        # issue wait for wr DMA #(k+9)'s completion -- scheduler-proof
        # in-ring priority: the z path gets the fabric first, u cascades
        # in as wr retires. u7-9 are emitted lazily in the loop bodies
        # (their recycle waits resolve instantly there). ----
        uts = []

        def load_u(ci):
            ut = upool.tile([SP, CW], F32, tag="u", name="u")
            nc.gpsimd.dma_start(
                ut[:].rearrange("p (c t) -> p c t", c=CH),
                din["u"][:, ci * CH:(ci + 1) * CH, :])
            uts.append(ut)

        for ci in range(3):
            load_u(ci)

        ones = const.tile([65, 128], BF16, tag="ones", name="ones")
        nc.vector.memset(ones[:], 1.0)
        ones16 = const.tile([65, 128], F16, tag="ones16", name="ones16")
        nc.vector.memset(ones16[:], 1.0)

        # ---------- gumbel front-run ----------
        def gumbel_chunk(ci):
            ut = uts[ci]
            nc.scalar.activation(ut[:], ut[:],
                                 mybir.ActivationFunctionType.Ln)
            nc.vector.reciprocal_approx_fast(ut[:], ut[:])

        for ci in range(3):
            gumbel_chunk(ci)

        # ---------- GCN (bn folded into adjT; K-groups are 4p+k) ----
        def lrelu_from_psum(ps_ap, out_tile, width):
            tmp = small.tile([width, T], F32, tag=f"lr{width}",
                             name=f"lr{width}")
            nc.vector.tensor_scalar_mul(tmp[:], ps_ap, NEG_SLOPE)
            nc.vector.tensor_tensor(out_tile[:], tmp[:], ps_ap,
                                    op=mybir.AluOpType.max)

        xw1 = [small.tile([125, H1], BF16, tag=f"xw1{g}", name=f"xw1{g}")
               for g in range(KT)]
        for g in range(KT):
            ps = psum.tile([125, H1], F32, tag="ps_small", name="ps_small")
            for k in range(KT):
                nc.tensor.matmul(ps[:], xT(k, g), w1(k),
                                 start=(k == 0), stop=(k == KT - 1))
            nc.vector.tensor_copy(xw1[g][:], ps[:])

        a1ps = psum.tile([H1, T], F32, tag="ps_small", name="ps_small")
        for k in range(KT):
            nc.tensor.matmul(a1ps[:], xw1[k][:], adjT(k),
                             start=(k == 0), stop=False)
        nc.tensor.matmul(a1ps[:], b1[:], grow[:], start=False, stop=False)
        nc.tensor.matmul(a1ps[:], ones[0:1, :H1], brow[:], start=False,
                         stop=True)
        h1T = small.tile([H1, T], BF16, tag="h1T", name="h1T")
        lrelu_from_psum(a1ps[:], h1T, H1)

        xw2 = [small.tile([125, H2], BF16, tag=f"xw2{g}", name=f"xw2{g}")
               for g in range(KT)]
        for g in range(KT):
            ps = psum.tile([125, H2], F32, tag="ps_small", name="ps_small")
            nc.tensor.matmul(ps[:], h1T[:, g::KT], w2[:],
                             start=True, stop=True)
            nc.vector.tensor_copy(xw2[g][:], ps[:])

        a2ps = psum.tile([H2, T], F32, tag="ps_small", name="ps_small")
        for k in range(KT):
            nc.tensor.matmul(a2ps[:], xw2[k][:], adjT(k),
                             start=(k == 0), stop=False)
        nc.tensor.matmul(a2ps[:], b2[:], grow[:], start=False, stop=False)
        nc.tensor.matmul(a2ps[:], ones[0:1, :H2], brow[:], start=False,
                         stop=True)
        h2T = small.tile([H2, T], BF16, tag="h2T", name="h2T")
        lrelu_from_psum(a2ps[:], h2T, H2)

        h2b = [small.tile([125, H2], BF16, tag=f"h2b{k}", name=f"h2b{k}")
               for k in range(KT)]
        for k in range(KT):
            pt = psum.tile([125, H2], BF16, tag="ps_tr", name="ps_tr")
            nc.tensor.transpose(pt[:], h2T[:, k::KT], ident[:])
            nc.vector.tensor_copy(h2b[k][:], pt[:])

        # ---------- z = xf @ actgen_w (bf16 stream) ----------
        zps = psum.tile([1, T], F32, tag="ps_z", name="ps_z")
        first = True
        for c in range(H2):
            wt = wts[c // 2]
            off = (c % 2) * KT * T
            for k in range(KT):
                nc.tensor.matmul(zps[:], h2b[k][:, c:c + 1],
                                 wt[:, off + k * T:off + (k + 1) * T],
                                 start=first,
                                 stop=(c == H2 - 1 and k == KT - 1))
                first = False
        zrow = small.tile([1, T], F32, tag="zrow", name="zrow")
        nc.vector.tensor_copy(zrow[:], zps[:])
        zrow16 = small.tile([1, T], F16, tag="zrow16", name="zrow16")
        nc.vector.tensor_copy(zrow16[:], zrow[:])

        # ---------- logits;  L = exp(logits) in fp16 ----------
        lgp = psum.tile([R, T], F32, tag="ps_lg", name="ps_lg")
        for k in range(KT):
            nc.tensor.matmul(lgp[:], dclT(k), av(k),
                             start=(k == 0), stop=False)
        nc.tensor.matmul(lgp[:], ones16[0:1, :R], zrow16[:], start=False,
                         stop=True)
        logits = small.tile([R, T], F32, tag="logits", name="logits")
        nc.scalar.activation(logits[:], lgp[:],
                             mybir.ActivationFunctionType.Exp)
        lg16 = small.tile([R, T], F16, tag="lg16", name="lg16")
        nc.vector.tensor_copy(lg16[:], logits[:])

        # Pack the 50 L rows into 3 lanes at base partitions 0/32/64 (the
        # only legal matmul operand bases) via a DRAM bounce (scalar ring).
        LPL = 17
        # bounce rides the sync ring: its sems retired with the small
        # params long ago, so these fire the moment exp lands instead of
        # queueing behind the u cascade on the gpsimd ring
        ld = dram.tile([R, T], F16, name="ldram")
        nc.scalar.dma_start(ld[:], lg16[:])
        lgflat = small.tile([65, LPL * T], F16, tag="lgflat", name="lgflat")
        nc.scalar.dma_start(
            lgflat[0:33:32, :].rearrange("l (j t) -> l j t", j=LPL),
            ld[0:2 * LPL].rearrange("(l j) t -> l j t", l=2))
        nc.scalar.dma_start(
            lgflat[64:65, :(R - 2 * LPL) * T],
            ld[2 * LPL:R].rearrange("(o j) t -> o (j t)", o=1))

        def lg_slice(r):
            lane, j = r // LPL, r % LPL
            return (lgflat[lane * 32:lane * 32 + 1, j * T:(j + 1) * T],
                    ones16[lane * 32:lane * 32 + 1, :SP])

        # ---------- main sampling loop ----------
        for ci in range(NCHUNK):
            ut = uts[ci]
            ot = opool.tile([SP, CW], F16, tag="o", name="o")
            for g in range(CH):
                r = ci * CH + g
                seg = slice(g * T, (g + 1) * T)
                rhs, lhs_ones = lg_slice(r)
                bt = bppool.tile([SP, 512], F32, tag="bp", name="bp")
                nc.tensor.matmul(bt[:, :T], lhs_ones, rhs,
                                 start=True, stop=True)
                qt = qpool.tile([SP, T], F32, tag="q", name="q")
                ss = spool.tile([SP, 1], F32, tag="ss", name="ss")
                nc.vector.scalar_tensor_tensor(
                    qt[:], bt[:, :T], 0.0, ut[:, seg],
                    op0=mybir.AluOpType.bypass, op1=mybir.AluOpType.mult,
                    accum_out=ss[:])
                nc.gpsimd.normalize_recip(ot[:, seg], qt[:], ss[:])
            # store rides the gpsimd ring right behind this chunk's
            # normalizes (producer order); lazy u issues keep the sync
            # ring's buf-recycle waits out of everyone's way
            # stores ride the sync ring (idle after the bounce; its sems
            # are long retired) so they never perturb the u cascade
            nc.sync.dma_start(out[:, ci * CH:(ci + 1) * CH, :],
                                ot[:].rearrange("p (c t) -> p c t", c=CH))
            if ci + 3 < NCHUNK:
                load_u(ci + 3)
                gumbel_chunk(ci + 3)


def _get_nc():
    if "nc" not in _CACHE:
        _CACHE["nc"] = _build()
    return _CACHE["nc"]


def prep_in_maps(inputs):
    f32 = np.float32
    f16 = np.float16
    import ml_dtypes
    state = np.asarray(inputs["state"], f32)[0]
    payoff = np.asarray(inputs["payoff"], f32)
    noise = np.asarray(inputs["feat_noise"], f32)[0]
    xT = np.concatenate([state, payoff, noise], axis=1).T  # (504, 500)
    gamma = np.asarray(inputs["bn_gamma"], f32)
    beta = np.asarray(inputs["bn_beta"], f32)
    adjT = (np.asarray(inputs["norm_adj"], f32) * gamma[:, None]).T
    dclT = np.asarray(inputs["def_cur_loc"], f32).T      # (500, 50)
    av = np.asarray(inputs["actgen_v"], f32)             # (500, 500)
    wr_full = np.asarray(inputs["actgen_w"], f32).reshape(T, H2, T)
    wr_full = np.ascontiguousarray(wr_full.transpose(1, 0, 2))

    def pack4(a, p):  # [4p, N] -> [p, 4N]; partition row i holds 4i..4i+3
        n = a.shape[1]
        return np.ascontiguousarray(a).reshape(p, 4 * n)

    bf16 = ml_dtypes.bfloat16
    pk32 = np.zeros((126, PK32_W), bf16)
    pk32[:, PK_XT:PK_XT + KT * T] = pack4(xT, 126).astype(bf16)
    pk32[:125, PK_ADJ:PK_ADJ + KT * T] = pack4(adjT, 125).astype(bf16)
    pk32[:, PK_W1:PK_W1 + KT * H1] = pack4(
        np.asarray(inputs["gc1_w"], f32), 126).astype(bf16)
    pk16 = np.zeros((125, PK16_W), f16)
    pk16[:, PK_AV:PK_AV + KT * T] = pack4(av, 125).astype(f16)
    pk16[:, PK_DCL:PK_DCL + KT * R] = pack4(dclT, 125).astype(f16)

    common = {
        "pk32": pk32,
        "pk16": pk16,
        "b1": np.asarray(inputs["gc1_b"], f32).reshape(1, H1).astype(bf16),
        "w2": np.asarray(inputs["gc2_w"], f32).astype(bf16),
        "b2": np.asarray(inputs["gc2_b"], f32).reshape(1, H2).astype(bf16),
        "grow": gamma.reshape(1, T).astype(bf16),
        "brow": beta.reshape(1, T).astype(bf16),
        "ident": np.eye(H2).astype(bf16),
        "wr": np.ascontiguousarray(
            wr_full.reshape(H2 // 2, 2, 125, KT, T).transpose(0, 2, 1, 3, 4)
            .reshape(H2 // 2, 125, 2 * KT * T)).astype(ml_dtypes.bfloat16),
    }
    u = np.asarray(inputs["gumbel_u"], f32)
    in_maps = []
    for i in range(NCORES):
        m = dict(common)
        m["u"] = np.ascontiguousarray(u[i * SP:(i + 1) * SP])
        in_maps.append(m)
    return in_maps


def run(inputs, trace=False):
    nc = _get_nc()
    in_maps = prep_in_maps(inputs)
    res = bass_utils.run_bass_kernel_spmd(
        nc, in_maps, core_ids=list(range(NCORES)), trace=trace)
    full = np.concatenate([res.results[i]["out"] for i in range(NCORES)],
                          axis=0).astype(np.float32)
    return full, res


def kernel(**inputs):
    full, _ = run(inputs)
    return full


# revision 22
# speedup vs baseline: 1.0888x; 1.0112x over previous
"""Trainium2 Bass kernel for nn_Def_A2C_Sample_Generator.

Computation (see reference):
  x = concat(state, payoff, noise)            (500, 504)
  h1 = lrelu(bn(adj @ (x @ w1) + b1))         (500, 32)
  h2 = lrelu(bn(adj @ (h1 @ w2) + b2))        (500, 16)
  xf = h2.reshape(8000)
  logits = xf @ actgen_w + def_cur_loc @ actgen_v          (50, 500)
  out = softmax(logits[None] + gumbel(u), axis=-1)         (1000, 50, 500)

Sharding: data-parallel over the 1000 samples, 125 per core on 8
cores. Each core computes the logits redundantly and softmaxes its own
125 x 50 x 500 gumbel block.

Softmax factoring: with v = ln(u) (ONE chunk-wide ACT pass),
  exp(logits + g) = L * exp(g) = L * (-1/v),  L = exp(logits)
and the minus sign cancels in the softmax normalization, so
  out = (L/v) / sum_t(L/v)
  v    : Ln ACT pass, chunk-wide in-place          (scalar engine)
  a    : reciprocal_approx_fast(v) in-place        (DVE, ~51 ULP)
  L    : PE ones-matmul broadcast of the fp16 L row into PSUM
  q,S  : DVE scalar_tensor_tensor mult, fused row-sum accum
  out  : gpsimd normalize_recip (out = q / S)      (GPSIMD engine)

DMA model (measured): the fabric sustains ~210GB/s per core TOTAL
(all 8 SPMD cores share chip HBM) and round-robins it across ALL
outstanding DMAs, so bytes are the floor and the only priority knob is
who is outstanding. Each ring throttles itself through its own ~8-deep
recycled semaphore pool, which IS scheduler-proof in-ring ordering:
  gpsimd ring : bf16 param pack, actgen_w as 8 channel-pair DMAs
                (8KB contiguous descriptors), then the u chunk loads --
                each u DMA's issue waits (via sem recycling) for an
                earlier wr DMA's completion, so the z/logits path gets
                the fabric first and u cascades in as wr retires.
  scalar ring : f16 pack + small params + the logits bounce (its sems
                retire early, so the bounce fires the moment exp lands).
  sync ring   : per-chunk output stores (fp16, [125, 5000] = 20KB
                contiguous per partition), kept off the u cascade.
Byte diet: u stays f32 (ln() near u=1 needs the mantissa) but params
ship bf16/f16 and the OUTPUT is stored fp16 and upcast on the host
(~5e-4 rel err, 2e-3 budget) -- 27MB of traffic instead of 36MB.
All [500,*] DRAM params ship packed 4-rows-per-partition (partition p
holds rows 4p..4p+3 contiguously); matmuls use stride-4 stationary
column slices so K-groupings of stationary and moving operands match.
"""
import sys

if "/opt/trn_rl_repo" not in sys.path:
    sys.path.insert(0, "/opt/trn_rl_repo")

import numpy as np

import concourse.bacc as bacc
import concourse.bass as bass
import concourse.mybir as mybir
import concourse.tile as tile
from concourse import bass_utils

# Keep every Exp and Ln on act-table set 6 (natural_log_exp_and_others)
# so one table load serves both and nothing thrashes.
_orig_get_act_tables = bacc.get_activation_tables


def _patched_get_act_tables(arch):
    tabs = dict(_orig_get_act_tables(arch))
    both = {mybir.ActivationFunctionType.Exp, mybir.ActivationFunctionType.Ln}
    for name, fns in tabs.items():
        if name != "natural_log_exp_and_others" and (both & fns):
            tabs[name] = fns - both
    return tabs


bacc.get_activation_tables = _patched_get_act_tables

F32 = mybir.dt.float32
F16 = mybir.dt.float16
BF16 = mybir.dt.bfloat16
NCORES = 8
T = 500
R = 50
NS = 1000
SP = NS // NCORES  # 125 samples per core
H1, H2 = 32, 16
FIN = 504
KT = 4
NEG_SLOPE = 0.2
CH = 10  # r's per chunk (20KB contiguous per partition per DMA)
CW = CH * T
NCHUNK = R // CH  # 10
FRONT = 2  # chunks 0-1 front-run; rest emitted lazily in-loop

# f32 param pack column offsets: xT | adjT | w1
PK_XT, PK_ADJ, PK_W1 = 0, KT * T, 2 * KT * T
PK32_W = 2 * KT * T + KT * H1  # 4128
# f16 param pack: av | dclT
PK_AV, PK_DCL = 0, KT * T
PK16_W = KT * T + KT * R  # 2200

_CACHE = {}


def _build():
    nc = bacc.Bacc("TRN2", target_bir_lowering=False, debug=False,
                   enable_asserts=False, num_devices=NCORES)

    din = {}
    din["pk32"] = nc.dram_tensor("pk32", [126, PK32_W], BF16,
                                 kind="ExternalInput")
    din["pk16"] = nc.dram_tensor("pk16", [125, PK16_W], F16,
                                 kind="ExternalInput")
    din["b1"] = nc.dram_tensor("b1", [1, H1], BF16, kind="ExternalInput")
    din["w2"] = nc.dram_tensor("w2", [H1, H2], BF16, kind="ExternalInput")
    din["b2"] = nc.dram_tensor("b2", [1, H2], BF16, kind="ExternalInput")
    din["grow"] = nc.dram_tensor("grow", [1, T], BF16, kind="ExternalInput")
    din["brow"] = nc.dram_tensor("brow", [1, T], BF16, kind="ExternalInput")
    din["ident"] = nc.dram_tensor("ident", [H2, H2], BF16,
                                  kind="ExternalInput")
    din["wr"] = nc.dram_tensor("wr", [H2 // 2, 125, 2 * KT * T], BF16,
                               kind="ExternalInput")
    din["u"] = nc.dram_tensor("u", [SP, R, T], F32, kind="ExternalInput")
    out = nc.dram_tensor("out", [SP, R, T], F16, kind="ExternalOutput")

    with tile.TileContext(nc) as tc:
        _emit(nc, tc, din, out)
    nc.compile()
    return nc


def _emit(nc, tc, din, out):
    from contextlib import ExitStack

    ctx = ExitStack()
    with ctx:
        # ---------- pools ----------
        const = ctx.enter_context(tc.tile_pool(name="const", bufs=1))
        small = ctx.enter_context(tc.tile_pool(name="small", bufs=1))
        psum = ctx.enter_context(tc.tile_pool(name="psum", bufs=1,
                                              space="PSUM"))
        dram = ctx.enter_context(tc.tile_pool(name="dram", bufs=1,
                                              space="DRAM"))
        upool = ctx.enter_context(tc.tile_pool(name="upool", bufs=3))
        opool = ctx.enter_context(tc.tile_pool(name="opool", bufs=3))
        qpool = ctx.enter_context(tc.tile_pool(name="qpool", bufs=3))
        spool = ctx.enter_context(tc.tile_pool(name="spool", bufs=8))
        bppool = ctx.enter_context(tc.tile_pool(name="bppool", bufs=4,
                                                space="PSUM"))
        wpool = ctx.enter_context(tc.tile_pool(name="wpool", bufs=8))

        # ---------- params: 2 packed DMAs + small stragglers ----------
        pk32 = const.tile([126, PK32_W], BF16, tag="pk32", name="pk32")
        nc.gpsimd.dma_start(pk32[:], din["pk32"][:])
        pk16 = const.tile([125, PK16_W], F16, tag="pk16", name="pk16")
        nc.scalar.dma_start(pk16[:], din["pk16"][:])
        b1 = const.tile([1, H1], BF16, tag="b1", name="b1")
        nc.scalar.dma_start(b1[:], din["b1"][:])
        w2 = const.tile([H1, H2], BF16, tag="w2", name="w2")
        nc.scalar.dma_start(w2[:], din["w2"][:])
        b2 = const.tile([1, H2], BF16, tag="b2", name="b2")
        nc.scalar.dma_start(b2[:], din["b2"][:])
        grow = const.tile([1, T], BF16, tag="grow", name="grow")
        nc.scalar.dma_start(grow[:], din["grow"][:])
        brow = const.tile([1, T], BF16, tag="brow", name="brow")
        nc.scalar.dma_start(brow[:], din["brow"][:])
        ident = const.tile([H2, H2], BF16, tag="ident", name="ident")
        nc.scalar.dma_start(ident[:], din["ident"][:])

        def xT(k, g=None):  # f-block k, target group g (cols 4m+g)
            base = PK_XT + k * T
            if g is None:
                return pk32[:, base:base + T]
            return pk32[:, base + g:base + T:KT]

        def adjT(k):
            base = PK_ADJ + k * T
            return pk32[0:125, base:base + T]

        def w1(k):
            base = PK_W1 + k * H1
            return pk32[:, base:base + H1]

        def av(k):
            return pk16[:, PK_AV + k * T:PK_AV + (k + 1) * T]

        def dclT(k):
            return pk16[:, PK_DCL + k * R:PK_DCL + (k + 1) * R]

        # ---------- wr stream: 16 un-gated DMAs emitted right after the
        # params. The global DMA semaphore pool (~11 sems, recycled in
        # emission order) makes DMA #n wait for #(n-11)'s completion, so
        # the u loads emitted after these auto-cascade in as wr DMAs
        # retire -- scheduler-proof bandwidth priority for the z path. --
        wts = []
        for g in range(H2 // 2):
            wt = wpool.tile([125, 2 * KT * T], BF16, tag="wr", name="wr")
            nc.gpsimd.dma_start(wt[:], din["wr"][g])
            wts.append(wt)

        # ---------- u loads: SAME ring as wr, directly behind it. The
        # gpsimd ring's sem pool (~8, recycled in ring order) makes u_k's
        # issue wait for wr DMA #(k+9)'s completion -- scheduler-proof
        # in-ring priority: the z path gets the fabric first, u cascades
        # in as wr retires. u7-9 are emitted lazily in the loop bodies
        # (their recycle waits resolve instantly there). ----
        uts = []

        def load_u(ci):
            ut = upool.tile([SP, CW], F32, tag="u", name="u")
            nc.gpsimd.dma_start(
                ut[:].rearrange("p (c t) -> p c t", c=CH),
                din["u"][:, ci * CH:(ci + 1) * CH, :])
            uts.append(ut)

        for ci in range(3):
            load_u(ci)

        ones = const.tile([65, 128], BF16, tag="ones", name="ones")
        nc.vector.memset(ones[:], 1.0)
        ones16 = const.tile([65, 128], F16, tag="ones16", name="ones16")
        nc.vector.memset(ones16[:], 1.0)

        # ---------- gumbel front-run ----------
        def gumbel_chunk(ci):
            ut = uts[ci]
            nc.scalar.activation(ut[:], ut[:],
                                 mybir.ActivationFunctionType.Ln)
            nc.vector.reciprocal_approx_fast(ut[:], ut[:])

        for ci in range(3):
            gumbel_chunk(ci)

        # ---------- GCN (bn folded into adjT; K-groups are 4p+k) ----
        def lrelu_from_psum(ps_ap, out_tile, width):
            tmp = small.tile([width, T], F32, tag=f"lr{width}",
                             name=f"lr{width}")
            nc.vector.tensor_scalar_mul(tmp[:], ps_ap, NEG_SLOPE)
            nc.vector.tensor_tensor(out_tile[:], tmp[:], ps_ap,
                                    op=mybir.AluOpType.max)

        xw1 = [small.tile([125, H1], BF16, tag=f"xw1{g}", name=f"xw1{g}")
               for g in range(KT)]
        for g in range(KT):
            ps = psum.tile([125, H1], F32, tag="ps_small", name="ps_small")
            for k in range(KT):
                nc.tensor.matmul(ps[:], xT(k, g), w1(k),
                                 start=(k == 0), stop=(k == KT - 1))
            nc.vector.tensor_copy(xw1[g][:], ps[:])

        a1ps = psum.tile([H1, T], F32, tag="ps_small", name="ps_small")
        for k in range(KT):
            nc.tensor.matmul(a1ps[:], xw1[k][:], adjT(k),
                             start=(k == 0), stop=False)
        nc.tensor.matmul(a1ps[:], b1[:], grow[:], start=False, stop=False)
        nc.tensor.matmul(a1ps[:], ones[0:1, :H1], brow[:], start=False,
                         stop=True)
        h1T = small.tile([H1, T], BF16, tag="h1T", name="h1T")
        lrelu_from_psum(a1ps[:], h1T, H1)

        xw2 = [small.tile([125, H2], BF16, tag=f"xw2{g}", name=f"xw2{g}")
               for g in range(KT)]
        for g in range(KT):
            ps = psum.tile([125, H2], F32, tag="ps_small", name="ps_small")
            nc.tensor.matmul(ps[:], h1T[:, g::KT], w2[:],
                             start=True, stop=True)
            nc.vector.tensor_copy(xw2[g][:], ps[:])

        a2ps = psum.tile([H2, T], F32, tag="ps_small", name="ps_small")
        for k in range(KT):
            nc.tensor.matmul(a2ps[:], xw2[k][:], adjT(k),
                             start=(k == 0), stop=False)
        nc.tensor.matmul(a2ps[:], b2[:], grow[:], start=False, stop=False)
        nc.tensor.matmul(a2ps[:], ones[0:1, :H2], brow[:], start=False,
                         stop=True)
        h2T = small.tile([H2, T], BF16, tag="h2T", name="h2T")
        lrelu_from_psum(a2ps[:], h2T, H2)

        h2b = [small.tile([125, H2], BF16, tag=f"h2b{k}", name=f"h2b{k}")
               for k in range(KT)]
        for k in range(KT):
            pt = psum.tile([125, H2], BF16, tag="ps_tr", name="ps_tr")
            nc.tensor.transpose(pt[:], h2T[:, k::KT], ident[:])
            nc.vector.tensor_copy(h2b[k][:], pt[:])

        # ---------- z = xf @ actgen_w (bf16 stream) ----------
        zps = psum.tile([1, T], F32, tag="ps_z", name="ps_z")
        first = True
        for c in range(H2):
            wt = wts[c // 2]
            off = (c % 2) * KT * T
            for k in range(KT):
                nc.tensor.matmul(zps[:], h2b[k][:, c:c + 1],
                                 wt[:, off + k * T:off + (k + 1) * T],
                                 start=first,
                                 stop=(c == H2 - 1 and k == KT - 1))
                first = False
        zrow = small.tile([1, T], F32, tag="zrow", name="zrow")
        nc.vector.tensor_copy(zrow[:], zps[:])
        zrow16 = small.tile([1, T], F16, tag="zrow16", name="zrow16")
        nc.vector.tensor_copy(zrow16[:], zrow[:])

        # ---------- logits;  L = exp(logits) in fp16 ----------
        lgp = psum.tile([R, T], F32, tag="ps_lg", name="ps_lg")
        for k in range(KT):
            nc.tensor.matmul(lgp[:], dclT(k), av(k),
                             start=(k == 0), stop=False)
        nc.tensor.matmul(lgp[:], ones16[0:1, :R], zrow16[:], start=False,
                         stop=True)
        logits = small.tile([R, T], F32, tag="logits", name="logits")
        nc.scalar.activation(logits[:], lgp[:],
                             mybir.ActivationFunctionType.Exp)
        lg16 = small.tile([R, T], F16, tag="lg16", name="lg16")
        nc.vector.tensor_copy(lg16[:], logits[:])

        # Pack the 50 L rows into 3 lanes at base partitions 0/32/64 (the
        # only legal matmul operand bases) via a DRAM bounce (scalar ring).
        LPL = 17
        # bounce rides the sync ring: its sems retired with the small
        # params long ago, so these fire the moment exp lands instead of
        # queueing behind the u cascade on the gpsimd ring
        ld = dram.tile([R, T], F16, name="ldram")
        nc.scalar.dma_start(ld[:], lg16[:])
        lgflat = small.tile([65, LPL * T], F16, tag="lgflat", name="lgflat")
        nc.scalar.dma_start(
            lgflat[0:33:32, :].rearrange("l (j t) -> l j t", j=LPL),
            ld[0:2 * LPL].rearrange("(l j) t -> l j t", l=2))
        nc.scalar.dma_start(
            lgflat[64:65, :(R - 2 * LPL) * T],
            ld[2 * LPL:R].rearrange("(o j) t -> o (j t)", o=1))

        def lg_slice(r):
            lane, j = r // LPL, r % LPL
            return (lgflat[lane * 32:lane * 32 + 1, j * T:(j + 1) * T],
                    ones16[lane * 32:lane * 32 + 1, :SP])

        # ---------- main sampling loop ----------
        for ci in range(NCHUNK):
            ut = uts[ci]
            ot = opool.tile([SP, CW], F16, tag="o", name="o")
            for g in range(CH):
                r = ci * CH + g
                seg = slice(g * T, (g + 1) * T)
                rhs, lhs_ones = lg_slice(r)
                bt = bppool.tile([SP, 512], F32, tag="bp", name="bp")
                nc.tensor.matmul(bt[:, :T], lhs_ones, rhs,
                                 start=True, stop=True)
                qt = qpool.tile([SP, T], F32, tag="q", name="q")
                ss = spool.tile([SP, 1], F32, tag="ss", name="ss")
                nc.vector.scalar_tensor_tensor(
                    qt[:], bt[:, :T], 0.0, ut[:, seg],
                    op0=mybir.AluOpType.bypass, op1=mybir.AluOpType.mult,
                    accum_out=ss[:])
                nc.gpsimd.normalize_recip(ot[:, seg], qt[:], ss[:])
            # store rides the gpsimd ring right behind this chunk's
            # normalizes (producer order); lazy u issues keep the sync
            # ring's buf-recycle waits out of everyone's way
            # stores ride the sync ring (idle after the bounce; its sems
            # are long retired) so they never perturb the u cascade
            nc.sync.dma_start(out[:, ci * CH:(ci + 1) * CH, :],
                                ot[:].rearrange("p (c t) -> p c t", c=CH))
            if ci + 3 < NCHUNK:
                load_u(ci + 3)
                gumbel_chunk(ci + 3)


def _get_nc():
    if "nc" not in _CACHE:
        _CACHE["nc"] = _build()
    return _CACHE["nc"]


def prep_in_maps(inputs):
    f32 = np.float32
    f16 = np.float16
    import ml_dtypes
    state = np.asarray(inputs["state"], f32)[0]
    payoff = np.asarray(inputs["payoff"], f32)
    noise = np.asarray(inputs["feat_noise"], f32)[0]
    xT = np.concatenate([state, payoff, noise], axis=1).T  # (504, 500)
    gamma = np.asarray(inputs["bn_gamma"], f32)
    beta = np.asarray(inputs["bn_beta"], f32)
    adjT = (np.asarray(inputs["norm_adj"], f32) * gamma[:, None]).T
    dclT = np.asarray(inputs["def_cur_loc"], f32).T      # (500, 50)
    av = np.asarray(inputs["actgen_v"], f32)             # (500, 500)
    wr_full = np.asarray(inputs["actgen_w"], f32).reshape(T, H2, T)
    wr_full = np.ascontiguousarray(wr_full.transpose(1, 0, 2))

    def pack4(a, p):  # [4p, N] -> [p, 4N]; partition row i holds 4i..4i+3
        n = a.shape[1]
        return np.ascontiguousarray(a).reshape(p, 4 * n)

    bf16 = ml_dtypes.bfloat16
    pk32 = np.zeros((126, PK32_W), bf16)
    pk32[:, PK_XT:PK_XT + KT * T] = pack4(xT, 126).astype(bf16)
    pk32[:125, PK_ADJ:PK_ADJ + KT * T] = pack4(adjT, 125).astype(bf16)
    pk32[:, PK_W1:PK_W1 + KT * H1] = pack4(
        np.asarray(inputs["gc1_w"], f32), 126).astype(bf16)
    pk16 = np.zeros((125, PK16_W), f16)
    pk16[:, PK_AV:PK_AV + KT * T] = pack4(av, 125).astype(f16)
    pk16[:, PK_DCL:PK_DCL + KT * R] = pack4(dclT, 125).astype(f16)

    common = {
        "pk32": pk32,
        "pk16": pk16,
        "b1": np.asarray(inputs["gc1_b"], f32).reshape(1, H1).astype(bf16),
        "w2": np.asarray(inputs["gc2_w"], f32).astype(bf16),
        "b2": np.asarray(inputs["gc2_b"], f32).reshape(1, H2).astype(bf16),
        "grow": gamma.reshape(1, T).astype(bf16),
        "brow": beta.reshape(1, T).astype(bf16),
        "ident": np.eye(H2).astype(bf16),
        "wr": np.ascontiguousarray(
            wr_full.reshape(H2 // 2, 2, 125, KT, T).transpose(0, 2, 1, 3, 4)
            .reshape(H2 // 2, 125, 2 * KT * T)).astype(ml_dtypes.bfloat16),
    }
    u = np.asarray(inputs["gumbel_u"], f32)
    in_maps = []
    for i in range(NCORES):
        m = dict(common)
        m["u"] = np.ascontiguousarray(u[i * SP:(i + 1) * SP])
        in_maps.append(m)
    return in_maps


def run(inputs, trace=False):
    nc = _get_nc()
    in_maps = prep_in_maps(inputs)
    res = bass_utils.run_bass_kernel_spmd(
        nc, in_maps, core_ids=list(range(NCORES)), trace=trace)
    full = np.concatenate([res.results[i]["out"] for i in range(NCORES)],
                          axis=0).astype(np.float32)
    return full, res


def kernel(**inputs):
    full, _ = run(inputs)
    return full
